# revision 1
# baseline (speedup 1.0000x reference)
"""Trainium2 Bass kernel for BlurredNoise: 128-filter 1D conv (K=5000) over
16 noise sequences, scaled per-filter.

Math: out[s, b, t] = sum_k noise[s, t+k] * F[b, k] * scale[b]
  s in [0,16) (= batch 2 x 8 noise channels), b in [0,128), t in [0,4096).

Mapping: data-parallel over the 16 sequences, 2 per NeuronCore. On each
core the conv is 40 accumulating 128x128x512 matmuls per output tile:
  k = 128*j + i,  lhsT_j[i, b] = F[b, 128j+i]*scale[b]  (prepped on host),
  rhs_j[i, t]    = X[t + 128j + i]   (slice of a Toeplitz band, host-built).
Operands are bf16 (fp32 PSUM accumulation): full PE rate with FWL weight
loads, measured rel-l2 error ~3e-3. Per-core: 640 matmuls at the 216 ns
warm pitch, input streaming double-ring'd and need-ordered so the PE
stream is gap-free; dead matmuls on a memset tile warm the HAM clock
gate during the first DMA's flight.
"""

import numpy as np
import ml_dtypes

import concourse.bacc as bacc
import concourse.mybir as mybir
from concourse.tile import TileContext
from concourse.bass_utils import run_bass_kernel_spmd

N_CORES = 8
BATCH = 2
NOISE_CH = 8
N_SEQ = BATCH * NOISE_CH          # 16
SEQ_PER_CORE = N_SEQ // N_CORES   # 2
T_IN = 9095
K_TAPS = 5000
T_OUT = 4096                      # T_IN - K_TAPS + 1
NJ = 40                           # ceil(5000/128)
K_PAD = NJ * 128                  # 5120
N_FILT = 128
NT = T_OUT // 512                 # 8 output tiles of 512
TAU = (NT - 1) * 512 + 512 + (NJ - 1) * 128   # 9088 Toeplitz band width
X_PAD = TAU + 128                 # 9216 >= 127 + 9087 + 1

_compiled_nc = None


def _build():
    nc = bacc.Bacc(name="blurred_noise")
    f32 = mybir.dt.float32
    bf16 = mybir.dt.bfloat16

    xt = nc.dram_tensor("xt", [SEQ_PER_CORE, 128, TAU], bf16, kind="ExternalInput")
    w = nc.dram_tensor("w", [128, K_PAD], bf16, kind="ExternalInput")
    out = nc.dram_tensor("out", [SEQ_PER_CORE, 128, T_OUT], f32, kind="ExternalOutput")

    with TileContext(nc) as tc:
        with (
            tc.tile_pool(name="wpool", bufs=1) as wp,
            tc.tile_pool(name="xpool", bufs=2) as xp,
            tc.tile_pool(name="opool", bufs=4) as op,
            tc.tile_pool(name="psum", bufs=8, space="PSUM") as pp,
        ):
            wt = wp.tile([128, K_PAD], bf16)
            xtiles = []
            for s in range(SEQ_PER_CORE):
                xs = xp.tile([128, TAU], bf16, name=f"xs{s}")
                xtiles.append(xs)
            # Inputs stream in chunks so the first matmuls only gate on the
            # first slabs (Tile tracks RAW deps at byte-range granularity).
            # Each HWDGE ring drains FIFO, so ordering transfers by first-use
            # IS the prioritization mechanism; the two head transfers (first
            # weight chunk on Scalar, first x band on Sync) land in parallel.
            for c0, c1 in ((0, 1536), (1536, 2560), (2560, 3584), (3584, 4608), (4608, K_PAD)):
                nc.scalar.dma_start(out=wt[:, c0:c1], in_=w[:, c0:c1])
            xloads = [
                (xtiles[0], xt[0], 0, 1536),
                (xtiles[0], xt[0], 1536, 3072),
                (xtiles[0], xt[0], 3072, 4608),
                (xtiles[0], xt[0], 4608, 6144),
                (xtiles[0], xt[0], 6144, 7040),
                (xtiles[0], xt[0], 7040, TAU),
                (xtiles[1], xt[1], 0, 2272),
                (xtiles[1], xt[1], 2272, 4544),
                (xtiles[1], xt[1], 4544, 6816),
                (xtiles[1], xt[1], 6816, TAU),
            ]
            for dst, src, c0, c1 in xloads:
                nc.sync.dma_start(out=dst[:, c0:c1], in_=src[:, c0:c1])
            # Warm the PE HAM clock-gate while the first x slabs stream in:
            # dead matmuls on a memset tile — no DMA dependency, so the PE
            # starts right after the preamble and is at 2.4 GHz by the time
            # the first real operands land.
            wsrc = op.tile([128, 256], bf16, name="wsrc")
            nc.gpsimd.memset(wsrc[:], 0.0)
            warm = pp.tile([128, 512], f32, name="warm", tag="acc")
            for i in range(40):
                nc.tensor.matmul(
                    warm[:, 0:128], wsrc[:, 0:128], wsrc[:, 128:256],
                    start=True, stop=True,
                )
            # Finer-grained trailing dummies: the handoff to the real stream
            # lands mid-dummy, so shorter dummies mean a smaller gap.
            for i in range(10):
                nc.tensor.matmul(
                    warm[:, 0:64], wsrc[:, 0:128], wsrc[:, 128:192],
                    start=True, stop=True,
                )
            # s1's tail groups narrow to 1-wide so the copy/DMA chain after
            # the very last matmul is as short as possible.
            groupings = {0: [(0, 4), (4, 4)], 1: [(0, 4), (4, 2), (6, 1), (7, 1)]}
            last = (SEQ_PER_CORE - 1, len(groupings[SEQ_PER_CORE - 1]) - 1)
            for s in range(SEQ_PER_CORE):
                for gi, (tbase, glen) in enumerate(groupings[s]):
                    ptiles = [
                        pp.tile([128, 512], f32, name=f"acc_{s}_{gi}_{i}", tag="acc")
                        for i in range(glen)
                    ]
                    def mm(j, tt):
                        col0 = (tbase + tt) * 512 + j * 128
                        nc.tensor.matmul(
                            ptiles[tt][:],
                            wt[:, j * 128:(j + 1) * 128],
                            xtiles[s][:, col0:col0 + 512],
                            start=(j == 0),
                            stop=(j == NJ - 1),
                        )

                    if s == 0 and gi == 0:
                        # PSUM banks accumulate independently, so front-load
                        # t0-t2 work (x chunk 1 only) while t3's first column
                        # window — which needs x chunk 2 — is still in flight.
                        for j in range(4):
                            for tt in range(glen - 1):
                                mm(j, tt)
                        for j in range(4):
                            mm(j, glen - 1)
                        for j in range(4, NJ):
                            for tt in range(glen):
                                mm(j, tt)
                    else:
                        for j in range(NJ):
                            for tt in range(glen):
                                mm(j, tt)
                    for tt in range(glen):
                        t0 = (tbase + tt) * 512
                        ot = op.tile([128, 512], f32)
                        if (s, gi) == last and tt == glen - 1:
                            # Half-copies let the first half's DMA launch
                            # while the second half is still copying.
                            nc.vector.tensor_copy(ot[:, 0:256], ptiles[tt][:, 0:256])
                            nc.scalar.dma_start(out=out[s][:, t0:t0 + 256], in_=ot[:, 0:256])
                            nc.vector.tensor_copy(ot[:, 256:512], ptiles[tt][:, 256:512])
                            nc.scalar.dma_start(out=out[s][:, t0 + 256:t0 + 512], in_=ot[:, 256:512])
                        else:
                            nc.vector.tensor_copy(ot[:], ptiles[tt][:])
                            nc.scalar.dma_start(out=out[s][:, t0:t0 + 512], in_=ot[:])
    nc.compile()
    return nc


def _get_nc():
    global _compiled_nc
    if _compiled_nc is None:
        _compiled_nc = _build()
    return _compiled_nc


def _prep_inputs(noise, blur_filters, output_scale):
    noise = np.ascontiguousarray(np.asarray(noise, dtype=np.float32))
    F = np.asarray(blur_filters, dtype=np.float32)
    scale = np.asarray(output_scale, dtype=np.float32).reshape(N_FILT)

    # Fold the per-filter output scale into the filters, zero-pad taps to 5120,
    # and lay out as W[i, 128j + b] = F[b, 128j + i] (contraction dim on
    # partitions, filter dim on the matmul free axis).
    gain = 1.0 + 1.0 * (scale - 1.0)
    Fp = np.zeros((N_FILT, K_PAD), dtype=np.float32)
    Fp[:, :K_TAPS] = F * gain[:, None]
    W = np.ascontiguousarray(
        Fp.reshape(N_FILT, NJ, 128).transpose(2, 1, 0).reshape(128, NJ * 128)
    ).astype(ml_dtypes.bfloat16)

    # Toeplitz band per sequence: band[s, i, tau] = X[s, i + tau].
    Xflat = np.zeros((N_SEQ, X_PAD), dtype=ml_dtypes.bfloat16)
    Xflat[:, :T_IN] = noise.reshape(N_SEQ, T_IN)
    sv = np.lib.stride_tricks.sliding_window_view(Xflat, TAU, axis=1)  # (16, 129, TAU)
    in_maps = []
    for c in range(N_CORES):
        xt = np.ascontiguousarray(
            sv[c * SEQ_PER_CORE:(c + 1) * SEQ_PER_CORE, :128, :]
        )  # (2, 128, TAU)
        in_maps.append({"xt": xt, "w": W})
    return in_maps


def _run(noise, blur_filters, output_scale, trace=False, tmpdir=None):
    in_maps = _prep_inputs(noise, blur_filters, output_scale)
    nc = _get_nc()
    res = run_bass_kernel_spmd(
        nc, in_maps, list(range(N_CORES)), trace=trace, tmpdir=tmpdir
    )
    outs = np.stack([res.results[c]["out"] for c in range(N_CORES)])  # (8, 2, 128, 4096)
    full = outs.reshape(BATCH, NOISE_CH, N_FILT, T_OUT).reshape(BATCH, NOISE_CH * N_FILT, T_OUT)
    return np.ascontiguousarray(full), res


def kernel(noise, blur_filters, output_scale):
    full, _ = _run(noise, blur_filters, output_scale)
    return full



# revision 2
# speedup vs baseline: 1.0433x; 1.0433x over previous
"""Trainium2 Bass kernel for BlurredNoise: 128-filter 1D conv (K=5000) over
16 noise sequences, scaled per-filter — ragged filter-streaming formulation.

Math: out[s, b, t] = sum_k noise[s, t+k] * F[b, k] * g[b]
  s in [0,16) (= batch 2 x 8 noise channels), b in [0,128), t in [0,4096).

The blur filters are banded: filter b has support only on taps
[5000-n_b, 5000) with n_b geometric from 125 to 5000, so the padded
128x5120 filter matrix is ~30% dense.  A dense formulation (filters
stationary, time streamed) must stream every 128-tap chunk at full time
width: 40 x 8 x 512 = 163,840 PE cycles/seq.  Here the X-Toeplitz block is
the stationary operand and the *filters* stream, so each chunk streams only
its active filter columns:

  psum[tau][t, b] += sum_i band[i, 128(tau+j) + t] * FT[i, 128j + b]
    band[i, c] = X[i + c]  (host-built Toeplitz band)
    FT[i, 128j + b] = F[b, 128j + i] * g[b]

PSUM layout: 8 banks x [128 t, 4 tau-slots x 128 b]; one matmul covers a
whole bank at diagonal s = tau + j via run j0 = s - 4B: chunks [j0-3, j0]
with a strided PSUM out AP [[128, nj], [1, aw]] (measured: free) and a
compacted rhs table holding only the aw = max-active window per run
(~55K streamed cols/seq of 164K dense).  lhsT = band[:, 128s:128s+128] is
shared by all banks at s; redundant LDWEIGHTS are stripped post-compile
(the PE keeps the stationary operand until the next load).

PSUM has_written: one start=True matmul per (seq, bank) writing the
always-covered last column of each slot clears the whole bank's bits and
orders (WAW) all later start=False accumulations, which then per-element
overwrite-on-first-touch / accumulate-after in any order.

Data-parallel over the 16 sequences, 2 per core.  Output lands
[t, b]-transposed in bf16; the host upcasts and transposes after gather.
"""

import json
import numpy as np
import ml_dtypes

import bass_rust
import concourse.bacc as bacc
import concourse.mybir as mybir
from concourse.tile import TileContext
from concourse.bass_utils import run_bass_kernel_spmd

N_CORES = 8
BATCH = 2
NOISE_CH = 8
N_SEQ = BATCH * NOISE_CH          # 16
SEQ_PER_CORE = N_SEQ // N_CORES   # 2
T_IN = 9095
K_TAPS = 5000
T_OUT = 4096
NJ = 40                           # tap chunks of 128
K_PAD = NJ * 128                  # 5120
N_FILT = 128
NTAU = T_OUT // 128               # 32 t-tiles
NBANK = NTAU // 4                 # 8 PSUM banks, 4 tau-slots each
NS = NTAU + NJ - 1                # 71 diagonals
NRUN = NJ + 3                     # 43 run tables (j0 = 0..42)
TAU_W = NS * 128                  # 9088
X_PAD = TAU_W + 128

_compiled = {}


# ---- post-compile BIR pass: drop redundant LDWEIGHTS ----------------------
def _ldw_sig(inst):
    d = {k: inst[k] for k in ("ins", "tile_position", "tile_size", "perf_mode",
                              "is_transpose") if k in inst}
    return json.dumps(d, sort_keys=True)


def _strip_redundant_ldweights(nc):
    """The PE array keeps the stationary operand until the next Ldweights, so
    a Ldweights identical to the previously executed one is a no-op that
    still costs weight-load pipe time.  Bass emits one per matmul; drop the
    repeats, moving any semaphore waits onto the next PE instruction."""
    obj = json.loads(bass_rust.module_to_json_string(nc.m))
    for fn in obj.get("functions", []):
        for bb in fn.get("blocks", []):
            out = []
            last_sig = None
            pending_waits = []
            for inst in bb.get("instructions", []):
                if inst.get("engine") != "PE":
                    out.append(inst)
                    continue
                op = inst.get("opcode")
                if op == "Ldweights":
                    si = inst.get("sync_info") or {}
                    if (_ldw_sig(inst) == last_sig
                            and not (si.get("on_update") or [])):
                        pending_waits.extend(si.get("on_wait") or [])
                        continue
                    last_sig = _ldw_sig(inst)
                elif op not in ("Matmult", "MatmultMx"):
                    last_sig = None
                if pending_waits:
                    si = inst.get("sync_info") or {"on_wait": [], "on_update": []}
                    si["on_wait"] = list(si.get("on_wait") or []) + pending_waits
                    si.setdefault("on_update", si.get("on_update") or [])
                    inst["sync_info"] = si
                    pending_waits = []
                out.append(inst)
            assert not pending_waits
            bb["instructions"] = out
    return bass_rust.module_from_json_string(json.dumps(obj))


# ---- kernel build ---------------------------------------------------------
def _runs_meta(a):
    meta = []
    off = 0
    for j0 in range(NRUN):
        jhi = min(NJ - 1, j0)
        jlo = max(0, j0 - 3)
        nj = jhi - jlo + 1
        aw = max(1, max(a[j] for j in range(jlo, jhi + 1)))
        meta.append((jhi, nj, aw, off))
        off += nj * aw
    return meta, off


def _build(a):
    meta, ftc_cols = _runs_meta(a)
    nc = bacc.Bacc(name="blurred_noise_rag4")
    f32 = mybir.dt.float32
    bf16 = mybir.dt.bfloat16

    xt = nc.dram_tensor("xt", [SEQ_PER_CORE, 128, TAU_W], bf16, kind="ExternalInput")
    w = nc.dram_tensor("w", [128, ftc_cols], bf16, kind="ExternalInput")
    out = nc.dram_tensor("out", [SEQ_PER_CORE, NTAU, 128, 128], bf16,
                         kind="ExternalOutput")

    with TileContext(nc) as tc:
        with (
            tc.tile_pool(name="wpool", bufs=1) as wp,
            tc.tile_pool(name="xpool", bufs=2) as xp,
            tc.tile_pool(name="opool", bufs=4) as op,
            tc.tile_pool(name="psum", bufs=8, space="PSUM") as pp,
        ):
            ftc = wp.tile([128, ftc_cols], bf16)
            bands = [xp.tile([128, TAU_W], bf16, name=f"band{s}")
                     for s in range(SEQ_PER_CORE)]

            # Stream inputs in first-use order, band0 alternating over the
            # sync and scalar HWDGE rings so it lands ~2x sooner. ftc run j0
            # is first used at diagonal s = j0; band cols [128s, 128s+128).
            f1 = meta[10][3]
            nf = 5
            fbounds = [0, f1] + [f1 + (ftc_cols - f1) * i // nf
                                 for i in range(1, nf + 1)]
            slabs = [(0, 384), (384, 1024), (1024, 1792), (1792, 2688),
                     (2688, 3712), (3712, 4736), (4736, 5760), (5760, 6912),
                     (6912, 8064), (8064, TAU_W)]
            nc.scalar.dma_start(out=ftc[:, 0:f1], in_=w[:, 0:f1])
            for k, (c0, c1) in enumerate(slabs):
                eng = nc.sync if k % 2 == 0 else nc.scalar
                eng.dma_start(out=bands[0][:, c0:c1], in_=xt[0][:, c0:c1])
            for c0, c1 in zip(fbounds[1:], fbounds[2:]):
                nc.scalar.dma_start(out=ftc[:, c0:c1], in_=w[:, c0:c1])
            for k, (c0, c1) in enumerate(slabs):
                eng = nc.sync if k % 2 == 0 else nc.scalar
                eng.dma_start(out=bands[1][:, c0:c1], in_=xt[1][:, c0:c1])

            # HAM warm-up on a zeroed tile while the first slabs stream in.
            zsrc = op.tile([128, 128], bf16, name="zsrc")
            nc.vector.memset(zsrc[:], 0.0)
            warm = pp.tile([128, 512], f32, name="warm", tag="acc")
            for i in range(24):
                nc.tensor.matmul(
                    warm[:, 0:128], zsrc[:, 0:128], zsrc[:, 0:128],
                    start=True, stop=True, skip_group_check=True,
                )

            banks = {}

            def bankfill(sq, b):
                # start=True clears the WHOLE bank's has_written bits; the
                # written region only needs to overlap every later matmul's
                # region (WAW ordering), and every run block ends at slot
                # column 128 — so 4 single columns suffice (N=4).
                t = pp.tile([128, 4, 128], f32, name=f"bank_{sq}_{b}", tag="acc")
                banks[(sq, b)] = t
                nc.tensor.matmul(
                    t[:, 0:4, 127:128], zsrc[:, 0:128], zsrc[:, 0:4],
                    start=True, stop=False, skip_group_check=True,
                )

            for b in range(3):
                bankfill(0, b)

            def copy_slot(sq, B, r):
                tau = 4 * B + r
                ot = op.tile([128, 128], bf16, name=f"ot_{sq}_{tau}")
                src = banks[(sq, B)][:, r, 0:128]
                if tau % 2 == 0:
                    nc.vector.tensor_copy(ot[:], src)
                    nc.gpsimd.dma_start(out=out[sq, tau], in_=ot[:])
                else:
                    nc.scalar.copy(ot[:], src)
                    nc.sync.dma_start(out=out[sq, tau], in_=ot[:])

            for sq in range(SEQ_PER_CORE):
                for s in range(NS):           # diagonals: s = 4B + j0 <= 70
                    for B in range(NBANK):
                        j0 = s - 4 * B
                        if j0 < 0 or j0 >= NRUN:
                            continue
                        if (sq, B) not in banks:
                            bankfill(sq, B)
                        jhi, nj, aw, off = meta[j0]
                        r_lo = j0 - jhi
                        nc.tensor.matmul(
                            banks[(sq, B)][:, r_lo:r_lo + nj, 128 - aw:128],
                            bands[sq][:, 128 * s:128 * (s + 1)],
                            ftc[:, off:off + nj * aw],
                            start=False, stop=(j0 == NRUN - 1),
                            skip_group_check=True,
                        )
                    # Copy a tau-slot only once its whole bank is done
                    # (s >= 4B + 42): a DVE/ACT read of a bank the PE is
                    # still writing stalls the PE (same-bank port conflict).
                    d = s - (NRUN - 1)
                    if 0 <= d:
                        B, r = d // 4, d % 4
                        if B < NBANK:
                            copy_slot(sq, B, r)
                for d in range(NS - (NRUN - 1), NTAU):
                    copy_slot(sq, d // 4, d % 4)
    nc.compile()
    nc.m = _strip_redundant_ldweights(nc)
    return nc


def _get_nc(a):
    key = tuple(a)
    if key not in _compiled:
        _compiled[key] = _build(a)
    return _compiled[key]


def _prep_inputs(noise, blur_filters, output_scale):
    noise = np.ascontiguousarray(np.asarray(noise, dtype=np.float32))
    F = np.asarray(blur_filters, dtype=np.float32)
    scale = np.asarray(output_scale, dtype=np.float32).reshape(N_FILT)

    gain = 1.0 + 1.0 * (scale - 1.0)
    Fp = np.zeros((N_FILT, K_PAD), dtype=np.float32)
    Fp[:, :K_TAPS] = F * gain[:, None]

    # Active-filter window per chunk derived from the data (filters sorted
    # by support length => active set of chunk j is [bm_j, 128)); dense or
    # unsorted filters degrade to full windows, staying correct.
    Fc = np.abs(Fp).reshape(N_FILT, NJ, 128).max(axis=2)  # [b, j]
    a = []
    for j in range(NJ):
        act = Fc[:, j] > 0
        if not act.any():
            a.append(1)
            continue
        first = int(np.argmax(act))
        if not act[first:].all():
            first = 0
        a.append(N_FILT - first)

    meta, ftc_cols = _runs_meta(a)
    # Compacted run table: run j0 holds chunks jhi..jlo (descending), each
    # restricted to the run's widest active window.
    FT = Fp.reshape(N_FILT, NJ, 128).transpose(2, 1, 0)  # [i, j, b]
    ftc = np.zeros((128, ftc_cols), dtype=np.float32)
    for j0 in range(NRUN):
        jhi, nj, aw, off = meta[j0]
        for idx in range(nj):
            j = jhi - idx
            ftc[:, off + idx * aw: off + (idx + 1) * aw] = FT[:, j, 128 - aw:]
    ftc = np.ascontiguousarray(ftc).astype(ml_dtypes.bfloat16)

    # Toeplitz band per sequence: band[s, i, c] = X[s, i + c].
    Xflat = np.zeros((N_SEQ, X_PAD), dtype=ml_dtypes.bfloat16)
    Xflat[:, :T_IN] = noise.reshape(N_SEQ, T_IN)
    sv = np.lib.stride_tricks.sliding_window_view(Xflat, TAU_W, axis=1)
    in_maps = []
    for c in range(N_CORES):
        xtc = np.ascontiguousarray(
            sv[c * SEQ_PER_CORE:(c + 1) * SEQ_PER_CORE, :128, :]
        )
        in_maps.append({"xt": xtc, "w": ftc})
    return in_maps, a


def _run(noise, blur_filters, output_scale, trace=False, tmpdir=None):
    in_maps, a = _prep_inputs(noise, blur_filters, output_scale)
    nc = _get_nc(a)
    res = run_bass_kernel_spmd(
        nc, in_maps, list(range(N_CORES)), trace=trace, tmpdir=tmpdir
    )
    outs = np.stack([np.asarray(res.results[c]["out"], dtype=np.float32)
                     for c in range(N_CORES)])
    # (8 cores, 2 seq, NTAU, 128 t, 128 b) -> (2, 1024, 4096)
    full = (
        outs.reshape(BATCH, NOISE_CH, NTAU, 128, N_FILT)
        .transpose(0, 1, 4, 2, 3)         # [n, c, b, tau, t]
        .reshape(BATCH, NOISE_CH * N_FILT, T_OUT)
    )
    return np.ascontiguousarray(full), res


def kernel(noise, blur_filters, output_scale):
    full, _ = _run(noise, blur_filters, output_scale)
    return full
